# revision 5
# baseline (speedup 1.0000x reference)
"""DeepSeek-V3 router kernel for Trainium2 (8 NeuronCores, SPMD).

Computes, for x:[8192,7168] f32, kernel:[7168,256] f32, bias:[256] f32:
    scores = sigmoid(x @ kernel)
    s = scores + bias
    group top-2 sums over 8 groups of 32 -> top-4 groups mask
    top-8 experts of masked s -> idx (int32), weights = normalized gathered
    sigmoid scores * 2.5
Returns (weights:[8192,8] f32, topk_idx:[8192,8] int32).

Sharding: x split along tokens across 8 cores (1024 tokens/core); router
weight + bias replicated. Host pre-tiles x into transposed layout so the
device GEMM needs no on-chip transpose.
"""
import sys

sys.path.insert(0, "/opt/trn_rl_repo")

import numpy as np

import concourse.bass as bass
import concourse.mybir as mybir
from concourse import bacc
from concourse.tile import TileContext
from concourse import bass_utils

T, D, E = 8192, 7168, 256
N_CORES = 8
TS = T // N_CORES          # tokens per core (1024)
NT = TS // 128             # token tiles per core (8)
KC = D // 128              # contraction chunks (56)
G, EPG = 8, 32             # expert groups, experts per group
TOPK_G, TOP_K = 4, 8
SCALE = 2.5
F32 = mybir.dt.float32
U32 = mybir.dt.uint32

_BUILt = {}


def build_nc(trace_scopes=False):
    nc = bacc.Bacc(None, target_bir_lowering=False)
    xp = nc.dram_tensor("xp", [NT, 128, KC, 128], F32, kind="ExternalInput")
    wk = nc.dram_tensor("wk", [128, KC, E], F32, kind="ExternalInput")
    bb = nc.dram_tensor("bb", [128, E], F32, kind="ExternalInput")
    wout = nc.dram_tensor("wout", [NT, 128, TOP_K], F32, kind="ExternalOutput")
    iout = nc.dram_tensor("iout", [NT, 128, TOP_K], U32, kind="ExternalOutput")

    with TileContext(nc) as tc:
        with (
            tc.tile_pool(name="const", bufs=1) as constp,
            tc.tile_pool(name="xin", bufs=2) as xinp,
            tc.tile_pool(name="ps", bufs=2, space="PSUM") as psp,
            tc.tile_pool(name="work", bufs=2) as workp,
            tc.tile_pool(name="small", bufs=2) as smallp,
        ):
            wk_sb = constp.tile([128, KC, E], F32)
            bb_sb = constp.tile([128, E], F32)
            nc.sync.dma_start(wk_sb, wk[:, :, :])
            nc.sync.dma_start(bb_sb, bb[:, :])

            for t in range(NT):
                xt = xinp.tile([128, KC, 128], F32, tag="xt")
                nc.sync.dma_start(xt, xp[t, :, :, :])

                acc = psp.tile([128, E], F32, tag="acc")
                for c in range(KC):
                    nc.tensor.matmul(acc, xt[:, c, :], wk_sb[:, c, :],
                                     start=(c == 0), stop=(c == KC - 1))

                # sigmoid on ACT (reads PSUM, writes SBUF)
                scores = workp.tile([128, E], F32, tag="scores")
                nc.scalar.activation(scores, acc,
                                     mybir.ActivationFunctionType.Sigmoid)
                # s = scores + bias
                s = workp.tile([128, E], F32, tag="s")
                nc.vector.tensor_add(s, scores, bb_sb)

                s3 = s[:].rearrange("p (g q) -> p g q", q=EPG)
                r1 = smallp.tile([128, G], F32, tag="r1")
                nc.vector.tensor_reduce(r1, s3, axis=mybir.AxisListType.X,
                                        op=mybir.AluOpType.max)
                mr = workp.tile([128, E], F32, tag="mr")
                nc.vector.match_replace(mr, r1, s, -1e30)
                r2 = smallp.tile([128, G], F32, tag="r2")
                nc.vector.tensor_reduce(
                    r2, mr[:].rearrange("p (g q) -> p g q", q=EPG),
                    axis=mybir.AxisListType.X, op=mybir.AluOpType.max)
                gs = smallp.tile([128, G], F32, tag="gs")
                nc.vector.tensor_add(gs, r1, r2)
                gs8 = smallp.tile([128, 8], F32, tag="gs8")
                nc.vector.max(gs8, gs)
                gmask = smallp.tile([128, G], F32, tag="gmask")
                nc.vector.tensor_scalar(gmask, gs, gs8[:, TOPK_G - 1:TOPK_G],
                                        None, op0=mybir.AluOpType.is_ge)
                # s_sel = s * gmask (mask broadcast over experts-per-group)
                s_sel = workp.tile([128, E], F32, tag="s_sel")
                nc.vector.tensor_mul(
                    s_sel[:].rearrange("p (g q) -> p g q", q=EPG), s3,
                    gmask[:].to_broadcast((128, G, EPG)))

                v8 = smallp.tile([128, 8], F32, tag="v8")
                nc.vector.max(v8, s_sel)
                i8 = smallp.tile([128, 8], U32, tag="i8")
                nc.vector.max_index(i8, v8, s_sel)

                # mark top-8 positions, build score array masked to them
                mark = workp.tile([128, E], F32, tag="mark")
                nc.vector.match_replace(mark, v8, s_sel, 2e30)
                hit = workp.tile([128, E], mybir.dt.uint8, tag="hit")
                nc.vector.tensor_scalar(hit, mark, 1e30, None,
                                        op0=mybir.AluOpType.is_ge)
                msc = workp.tile([128, E], F32, tag="msc")
                nc.vector.memset(msc, -1e30)
                nc.vector.copy_predicated(msc, hit, scores)
                sc8 = smallp.tile([128, 8], F32, tag="sc8")
                nc.vector.max(sc8, msc)
                isc8 = smallp.tile([128, 8], U32, tag="isc8")
                nc.vector.max_index(isc8, sc8, msc)

                # reorder sc8 (score-order) into s-rank order by index match
                i8f = smallp.tile([128, 8], F32, tag="i8f")
                nc.vector.tensor_copy(i8f, i8)
                isc8f = smallp.tile([128, 8], F32, tag="isc8f")
                nc.vector.tensor_copy(isc8f, isc8)
                terms = smallp.tile([128, 8, 8], F32, tag="terms")
                for k in range(8):
                    nc.vector.tensor_scalar(
                        terms[:, :, k], i8f, isc8f[:, k:k + 1], sc8[:, k:k + 1],
                        op0=mybir.AluOpType.is_equal, op1=mybir.AluOpType.mult)
                w8 = smallp.tile([128, 8], F32, tag="w8")
                nc.vector.tensor_reduce(w8, terms, axis=mybir.AxisListType.X,
                                        op=mybir.AluOpType.add)

                ssum = smallp.tile([128, 1], F32, tag="ssum")
                nc.vector.tensor_reduce(ssum, w8, axis=mybir.AxisListType.X,
                                        op=mybir.AluOpType.add)
                rec = smallp.tile([128, 1], F32, tag="rec")
                nc.vector.tensor_scalar(rec, ssum, 1e-20, None,
                                        op0=mybir.AluOpType.add)
                nc.vector.reciprocal(rec, rec)
                nc.vector.tensor_scalar(rec, rec, SCALE, None,
                                        op0=mybir.AluOpType.mult)
                wn = smallp.tile([128, 8], F32, tag="wn")
                nc.vector.tensor_scalar(wn, w8, rec[:, 0:1], None,
                                        op0=mybir.AluOpType.mult)

                nc.sync.dma_start(wout[t, :, :], wn)
                nc.sync.dma_start(iout[t, :, :], i8)
    nc.compile()
    return nc


def _prep_inputs(x, kernel, bias):
    """Host-side shard + retile. Returns list of 8 in_maps."""
    x = np.asarray(x, dtype=np.float32)
    kernel = np.asarray(kernel, dtype=np.float32)
    bias = np.asarray(bias, dtype=np.float32)
    wk = np.ascontiguousarray(
        kernel.reshape(KC, 128, E).transpose(1, 0, 2))          # [p, c, e]
    bb = np.ascontiguousarray(np.broadcast_to(bias, (128, E)))
    in_maps = []
    for core in range(N_CORES):
        xs = x[core * TS:(core + 1) * TS]                        # [1024, 7168]
        xp = np.ascontiguousarray(
            xs.reshape(NT, 128, KC, 128).transpose(0, 3, 2, 1))  # [t, p, c, j]
        in_maps.append({"xp": xp, "wk": wk, "bb": bb})
    return in_maps


def kernel(x, kernel, bias, _trace=False):
    if "nc" not in _BUILt:
        _BUILt["nc"] = build_nc()
    nc = _BUILt["nc"]
    in_maps = _prep_inputs(x, kernel, bias)
    res = bass_utils.run_bass_kernel_spmd(
        nc, in_maps, core_ids=list(range(N_CORES)), trace=_trace)
    weights = np.empty((T, TOP_K), np.float32)
    idx = np.empty((T, TOP_K), np.int32)
    for core in range(N_CORES):
        weights[core * TS:(core + 1) * TS] = res.results[core]["wout"].reshape(TS, TOP_K)
        idx[core * TS:(core + 1) * TS] = (
            res.results[core]["iout"].reshape(TS, TOP_K).astype(np.int32))
    _BUILt["last_result"] = res
    return weights, idx


# revision 6
# speedup vs baseline: 1.2616x; 1.2616x over previous
"""DeepSeek-V3 router kernel for Trainium2 (8 NeuronCores, SPMD).

Computes, for x:[8192,7168] f32, kernel:[7168,256] f32, bias:[256] f32:
    scores = sigmoid(x @ kernel)
    s = scores + bias
    group top-2 sums over 8 groups of 32 -> top-4 groups mask
    top-8 experts of masked s -> idx (int32), weights = normalized gathered
    sigmoid scores * 2.5
Returns (weights:[8192,8] f32, topk_idx:[8192,8] int32).

Sharding: x split along tokens across 8 cores (1024 tokens/core); router
weight + bias replicated. Host pre-tiles x into transposed layout so the
device GEMM needs no on-chip transpose.
"""
import sys

sys.path.insert(0, "/opt/trn_rl_repo")

import numpy as np

import concourse.bass as bass
import concourse.mybir as mybir
from concourse import bacc
from concourse.tile import TileContext
from concourse import bass_utils

T, D, E = 8192, 7168, 256
N_CORES = 8
TS = T // N_CORES          # tokens per core (1024)
NT = TS // 128             # token tiles per core (8)
KC = D // 128              # contraction chunks (56)
G, EPG = 8, 32             # expert groups, experts per group
TOPK_G, TOP_K = 4, 8
SCALE = 2.5
F32 = mybir.dt.float32
BF16 = mybir.dt.bfloat16
U32 = mybir.dt.uint32
WGRP = 8                   # kernel-chunk groups for pipelined weight loads

_BUILt = {}


def build_nc(trace_scopes=False):
    nc = bacc.Bacc(None, target_bir_lowering=False)
    xph = nc.dram_tensor("xph", [NT, 128, KC, 128], BF16, kind="ExternalInput")
    xpl = nc.dram_tensor("xpl", [NT, 128, KC, 128], BF16, kind="ExternalInput")
    wkh = nc.dram_tensor("wkh", [128, KC, E], BF16, kind="ExternalInput")
    wkl = nc.dram_tensor("wkl", [128, KC, E], BF16, kind="ExternalInput")
    bb = nc.dram_tensor("bb", [128, E], F32, kind="ExternalInput")
    wout = nc.dram_tensor("wout", [NT, 128, TOP_K], F32, kind="ExternalOutput")
    iout = nc.dram_tensor("iout", [NT, 128, TOP_K], U32, kind="ExternalOutput")

    with TileContext(nc) as tc:
        with (
            tc.tile_pool(name="const", bufs=1) as constp,
            tc.tile_pool(name="xin", bufs=2) as xinp,
            tc.tile_pool(name="ps", bufs=2, space="PSUM") as psp,
            tc.tile_pool(name="work", bufs=2) as workp,
            tc.tile_pool(name="small", bufs=2) as smallp,
        ):
            bb_sb = constp.tile([128, E], F32)
            nc.sync.dma_start(bb_sb, bb[:, :])
            gc = KC // WGRP  # chunks per weight group (7)
            wh_sb, wl_sb = [], []
            for g in range(WGRP):
                wh = constp.tile([128, gc, E], BF16, tag=f"wh{g}")
                wl = constp.tile([128, gc, E], BF16, tag=f"wl{g}")
                nc.sync.dma_start(wh, wkh[:, g * gc:(g + 1) * gc, :])
                nc.sync.dma_start(wl, wkl[:, g * gc:(g + 1) * gc, :])
                wh_sb.append(wh)
                wl_sb.append(wl)

            for t in range(NT):
                xh = xinp.tile([128, KC, 128], BF16, tag="xh")
                xl = xinp.tile([128, KC, 128], BF16, tag="xl")
                for g in range(WGRP):
                    nc.sync.dma_start(xh[:, g * gc:(g + 1) * gc, :],
                                      xph[t, :, g * gc:(g + 1) * gc, :])
                    nc.sync.dma_start(xl[:, g * gc:(g + 1) * gc, :],
                                      xpl[t, :, g * gc:(g + 1) * gc, :])

                acc = psp.tile([128, E], F32, tag="acc")
                for c in range(KC):
                    g = c // gc
                    whc = wh_sb[g][:, c - g * gc, :]
                    wlc = wl_sb[g][:, c - g * gc, :]
                    nc.tensor.matmul(acc, xh[:, c, :], whc,
                                     start=(c == 0), stop=False)
                    nc.tensor.matmul(acc, xh[:, c, :], wlc,
                                     start=False, stop=False)
                    nc.tensor.matmul(acc, xl[:, c, :], whc,
                                     start=False, stop=(c == KC - 1))

                # sigmoid on ACT (reads PSUM, writes SBUF)
                scores = workp.tile([128, E], F32, tag="scores")
                nc.scalar.activation(scores, acc,
                                     mybir.ActivationFunctionType.Sigmoid)
                # s = scores + bias
                s = workp.tile([128, E], F32, tag="s")
                nc.vector.tensor_add(s, scores, bb_sb)

                s3 = s[:].rearrange("p (g q) -> p g q", q=EPG)
                r1 = smallp.tile([128, G], F32, tag="r1")
                nc.vector.tensor_reduce(r1, s3, axis=mybir.AxisListType.X,
                                        op=mybir.AluOpType.max)
                mr = workp.tile([128, E], F32, tag="mr")
                nc.vector.match_replace(mr, r1, s, -1e30)
                r2 = smallp.tile([128, G], F32, tag="r2")
                nc.vector.tensor_reduce(
                    r2, mr[:].rearrange("p (g q) -> p g q", q=EPG),
                    axis=mybir.AxisListType.X, op=mybir.AluOpType.max)
                gs = smallp.tile([128, G], F32, tag="gs")
                nc.vector.tensor_add(gs, r1, r2)
                gs8 = smallp.tile([128, 8], F32, tag="gs8")
                nc.vector.max(gs8, gs)
                gmask = smallp.tile([128, G], F32, tag="gmask")
                nc.vector.tensor_scalar(gmask, gs, gs8[:, TOPK_G - 1:TOPK_G],
                                        None, op0=mybir.AluOpType.is_ge)
                # s_sel = s * gmask (mask broadcast over experts-per-group)
                s_sel = workp.tile([128, E], F32, tag="s_sel")
                nc.vector.tensor_mul(
                    s_sel[:].rearrange("p (g q) -> p g q", q=EPG), s3,
                    gmask[:].to_broadcast((128, G, EPG)))

                v8 = smallp.tile([128, 8], F32, tag="v8")
                nc.vector.max(v8, s_sel)
                i8 = smallp.tile([128, 8], U32, tag="i8")
                nc.vector.max_index(i8, v8, s_sel)

                # mark top-8 positions, build score array masked to them
                mark = workp.tile([128, E], F32, tag="mark")
                nc.vector.match_replace(mark, v8, s_sel, 2e30)
                hit = workp.tile([128, E], mybir.dt.uint8, tag="hit")
                nc.vector.tensor_scalar(hit, mark, 1e30, None,
                                        op0=mybir.AluOpType.is_ge)
                msc = workp.tile([128, E], F32, tag="msc")
                nc.vector.memset(msc, -1e30)
                nc.vector.copy_predicated(msc, hit, scores)
                sc8 = smallp.tile([128, 8], F32, tag="sc8")
                nc.vector.max(sc8, msc)
                isc8 = smallp.tile([128, 8], U32, tag="isc8")
                nc.vector.max_index(isc8, sc8, msc)

                # reorder sc8 (score-order) into s-rank order by index match
                i8f = smallp.tile([128, 8], F32, tag="i8f")
                nc.vector.tensor_copy(i8f, i8)
                isc8f = smallp.tile([128, 8], F32, tag="isc8f")
                nc.vector.tensor_copy(isc8f, isc8)
                terms = smallp.tile([128, 8, 8], F32, tag="terms")
                for k in range(8):
                    nc.vector.tensor_scalar(
                        terms[:, :, k], i8f, isc8f[:, k:k + 1], sc8[:, k:k + 1],
                        op0=mybir.AluOpType.is_equal, op1=mybir.AluOpType.mult)
                w8 = smallp.tile([128, 8], F32, tag="w8")
                nc.vector.tensor_reduce(w8, terms, axis=mybir.AxisListType.X,
                                        op=mybir.AluOpType.add)

                ssum = smallp.tile([128, 1], F32, tag="ssum")
                nc.vector.tensor_reduce(ssum, w8, axis=mybir.AxisListType.X,
                                        op=mybir.AluOpType.add)
                rec = smallp.tile([128, 1], F32, tag="rec")
                nc.vector.tensor_scalar(rec, ssum, 1e-20, None,
                                        op0=mybir.AluOpType.add)
                nc.vector.reciprocal(rec, rec)
                nc.vector.tensor_scalar(rec, rec, SCALE, None,
                                        op0=mybir.AluOpType.mult)
                wn = smallp.tile([128, 8], F32, tag="wn")
                nc.vector.tensor_scalar(wn, w8, rec[:, 0:1], None,
                                        op0=mybir.AluOpType.mult)

                nc.sync.dma_start(wout[t, :, :], wn)
                nc.sync.dma_start(iout[t, :, :], i8)
    nc.compile()
    return nc


def _prep_inputs(x, kernel, bias):
    """Host-side shard + retile. Returns list of 8 in_maps."""
    import ml_dtypes
    bf = ml_dtypes.bfloat16
    x = np.asarray(x, dtype=np.float32)
    kernel = np.asarray(kernel, dtype=np.float32)
    bias = np.asarray(bias, dtype=np.float32)
    kh = kernel.astype(bf)
    kl = (kernel - kh.astype(np.float32)).astype(bf)
    wkh = np.ascontiguousarray(
        kh.reshape(KC, 128, E).transpose(1, 0, 2))               # [p, c, e]
    wkl = np.ascontiguousarray(
        kl.reshape(KC, 128, E).transpose(1, 0, 2))
    bb = np.ascontiguousarray(np.broadcast_to(bias, (128, E)))
    in_maps = []
    for core in range(N_CORES):
        xs = x[core * TS:(core + 1) * TS]                        # [1024, 7168]
        xsh = xs.astype(bf)
        xsl = (xs - xsh.astype(np.float32)).astype(bf)
        xph = np.ascontiguousarray(
            xsh.reshape(NT, 128, KC, 128).transpose(0, 3, 2, 1))  # [t, p, c, j]
        xpl = np.ascontiguousarray(
            xsl.reshape(NT, 128, KC, 128).transpose(0, 3, 2, 1))
        in_maps.append({"xph": xph, "xpl": xpl, "wkh": wkh, "wkl": wkl,
                        "bb": bb})
    return in_maps


def kernel(x, kernel, bias, _trace=False):
    if "nc" not in _BUILt:
        _BUILt["nc"] = build_nc()
    nc = _BUILt["nc"]
    in_maps = _prep_inputs(x, kernel, bias)
    res = bass_utils.run_bass_kernel_spmd(
        nc, in_maps, core_ids=list(range(N_CORES)), trace=_trace)
    weights = np.empty((T, TOP_K), np.float32)
    idx = np.empty((T, TOP_K), np.int32)
    for core in range(N_CORES):
        weights[core * TS:(core + 1) * TS] = res.results[core]["wout"].reshape(TS, TOP_K)
        idx[core * TS:(core + 1) * TS] = (
            res.results[core]["iout"].reshape(TS, TOP_K).astype(np.int32))
    _BUILt["last_result"] = res
    return weights, idx


# revision 7
# speedup vs baseline: 1.2785x; 1.0134x over previous
"""DeepSeek-V3 router kernel for Trainium2 (8 NeuronCores, SPMD).

Computes, for x:[8192,7168] f32, kernel:[7168,256] f32, bias:[256] f32:
    scores = sigmoid(x @ kernel)
    s = scores + bias
    group top-2 sums over 8 groups of 32 -> top-4 groups mask
    top-8 experts of masked s -> idx (int32), weights = normalized gathered
    sigmoid scores * 2.5
Returns (weights:[8192,8] f32, topk_idx:[8192,8] int32).

Sharding: x split along tokens across 8 cores (1024 tokens/core); router
weight + bias replicated. Host pre-tiles x into transposed layout so the
device GEMM needs no on-chip transpose.
"""
import sys

sys.path.insert(0, "/opt/trn_rl_repo")

import numpy as np

import concourse.bass as bass
import concourse.mybir as mybir
from concourse import bacc
from concourse.tile import TileContext
from concourse import bass_utils

T, D, E = 8192, 7168, 256
N_CORES = 8
TS = T // N_CORES          # tokens per core (1024)
NT = TS // 128             # token tiles per core (8)
KC = D // 128              # contraction chunks (56)
G, EPG = 8, 32             # expert groups, experts per group
TOPK_G, TOP_K = 4, 8
SCALE = 2.5
F32 = mybir.dt.float32
F16 = mybir.dt.float16
SX, SW = 64.0, 1024.0
U32 = mybir.dt.uint32
WGRP = 8                   # kernel-chunk groups for pipelined weight loads

_BUILt = {}


def build_nc(trace_scopes=False):
    nc = bacc.Bacc(None, target_bir_lowering=False)
    xph = nc.dram_tensor("xph", [NT, 128, KC, 128], F16, kind="ExternalInput")
    xpl = nc.dram_tensor("xpl", [NT, 128, KC, 128], F16, kind="ExternalInput")
    wkh = nc.dram_tensor("wkh", [128, KC, E], F16, kind="ExternalInput")
    wkl = nc.dram_tensor("wkl", [128, KC, E], F16, kind="ExternalInput")
    bb = nc.dram_tensor("bb", [128, E], F32, kind="ExternalInput")
    wout = nc.dram_tensor("wout", [NT, 128, TOP_K], F32, kind="ExternalOutput")
    iout = nc.dram_tensor("iout", [NT, 128, TOP_K], U32, kind="ExternalOutput")

    with TileContext(nc) as tc:
        with (
            tc.tile_pool(name="const", bufs=1) as constp,
            tc.tile_pool(name="xin", bufs=2) as xinp,
            tc.tile_pool(name="ps", bufs=2, space="PSUM") as psp,
            tc.tile_pool(name="work", bufs=2) as workp,
            tc.tile_pool(name="small", bufs=2) as smallp,
        ):
            bb_sb = constp.tile([128, E], F32)
            nc.sync.dma_start(bb_sb, bb[:, :])
            gc = KC // WGRP  # chunks per weight group (7)
            wh_sb, wl_sb = [], []
            for g in range(WGRP):
                wh = constp.tile([128, gc, E], F16, tag=f"wh{g}")
                wl = constp.tile([128, gc, E], F16, tag=f"wl{g}")
                nc.sync.dma_start(wh, wkh[:, g * gc:(g + 1) * gc, :])
                nc.sync.dma_start(wl, wkl[:, g * gc:(g + 1) * gc, :])
                wh_sb.append(wh)
                wl_sb.append(wl)

            for t in range(NT):
                xh = xinp.tile([128, KC, 128], F16, tag="xh")
                xl = xinp.tile([128, KC, 128], F16, tag="xl")
                for g in range(WGRP):
                    nc.sync.dma_start(xh[:, g * gc:(g + 1) * gc, :],
                                      xph[t, :, g * gc:(g + 1) * gc, :])
                    nc.sync.dma_start(xl[:, g * gc:(g + 1) * gc, :],
                                      xpl[t, :, g * gc:(g + 1) * gc, :])

                acc = psp.tile([128, E], F32, tag="acc")
                for c in range(KC):
                    g = c // gc
                    whc = wh_sb[g][:, c - g * gc, :]
                    wlc = wl_sb[g][:, c - g * gc, :]
                    nc.tensor.matmul(acc, xh[:, c, :], whc,
                                     start=(c == 0), stop=False)
                    nc.tensor.matmul(acc, xh[:, c, :], wlc,
                                     start=False, stop=False)
                    nc.tensor.matmul(acc, xl[:, c, :], whc,
                                     start=False, stop=(c == KC - 1))

                # sigmoid on ACT (reads PSUM, writes SBUF)
                scores = workp.tile([128, E], F32, tag="scores")
                nc.scalar.activation(scores, acc,
                                     mybir.ActivationFunctionType.Sigmoid,
                                     scale=1.0 / (SX * SW))
                # s = scores + bias
                s = workp.tile([128, E], F32, tag="s")
                nc.vector.tensor_add(s, scores, bb_sb)

                s3 = s[:].rearrange("p (g q) -> p g q", q=EPG)
                r1 = smallp.tile([128, G], F32, tag="r1")
                nc.vector.tensor_reduce(r1, s3, axis=mybir.AxisListType.X,
                                        op=mybir.AluOpType.max)
                mr = workp.tile([128, E], F32, tag="mr")
                nc.vector.match_replace(mr, r1, s, -1e30)
                r2 = smallp.tile([128, G], F32, tag="r2")
                nc.vector.tensor_reduce(
                    r2, mr[:].rearrange("p (g q) -> p g q", q=EPG),
                    axis=mybir.AxisListType.X, op=mybir.AluOpType.max)
                gs = smallp.tile([128, G], F32, tag="gs")
                nc.vector.tensor_add(gs, r1, r2)
                gs8 = smallp.tile([128, 8], F32, tag="gs8")
                nc.vector.max(gs8, gs)
                gmask = smallp.tile([128, G], F32, tag="gmask")
                nc.vector.tensor_scalar(gmask, gs, gs8[:, TOPK_G - 1:TOPK_G],
                                        None, op0=mybir.AluOpType.is_ge)
                # s_sel = s * gmask (mask broadcast over experts-per-group)
                s_sel = workp.tile([128, E], F32, tag="s_sel")
                nc.vector.tensor_mul(
                    s_sel[:].rearrange("p (g q) -> p g q", q=EPG), s3,
                    gmask[:].to_broadcast((128, G, EPG)))

                v8 = smallp.tile([128, 8], F32, tag="v8")
                nc.vector.max(v8, s_sel)
                i8 = smallp.tile([128, 8], U32, tag="i8")
                nc.vector.max_index(i8, v8, s_sel)

                # mark top-8 positions, build score array masked to them
                mark = workp.tile([128, E], F32, tag="mark")
                nc.vector.match_replace(mark, v8, s_sel, 2e30)
                hit = workp.tile([128, E], mybir.dt.uint8, tag="hit")
                nc.vector.tensor_scalar(hit, mark, 1e30, None,
                                        op0=mybir.AluOpType.is_ge)
                msc = workp.tile([128, E], F32, tag="msc")
                nc.vector.memset(msc, -1e30)
                nc.vector.copy_predicated(msc, hit, scores)
                sc8 = smallp.tile([128, 8], F32, tag="sc8")
                nc.vector.max(sc8, msc)
                isc8 = smallp.tile([128, 8], U32, tag="isc8")
                nc.vector.max_index(isc8, sc8, msc)

                # reorder sc8 (score-order) into s-rank order by index match
                i8f = smallp.tile([128, 8], F32, tag="i8f")
                nc.vector.tensor_copy(i8f, i8)
                isc8f = smallp.tile([128, 8], F32, tag="isc8f")
                nc.vector.tensor_copy(isc8f, isc8)
                terms = smallp.tile([128, 8, 8], F32, tag="terms")
                for k in range(8):
                    nc.vector.tensor_scalar(
                        terms[:, :, k], i8f, isc8f[:, k:k + 1], sc8[:, k:k + 1],
                        op0=mybir.AluOpType.is_equal, op1=mybir.AluOpType.mult)
                w8 = smallp.tile([128, 8], F32, tag="w8")
                nc.vector.tensor_reduce(w8, terms, axis=mybir.AxisListType.X,
                                        op=mybir.AluOpType.add)

                ssum = smallp.tile([128, 1], F32, tag="ssum")
                nc.vector.tensor_reduce(ssum, w8, axis=mybir.AxisListType.X,
                                        op=mybir.AluOpType.add)
                rec = smallp.tile([128, 1], F32, tag="rec")
                nc.vector.tensor_scalar(rec, ssum, 1e-20, None,
                                        op0=mybir.AluOpType.add)
                nc.vector.reciprocal(rec, rec)
                nc.vector.tensor_scalar(rec, rec, SCALE, None,
                                        op0=mybir.AluOpType.mult)
                wn = smallp.tile([128, 8], F32, tag="wn")
                nc.vector.tensor_scalar(wn, w8, rec[:, 0:1], None,
                                        op0=mybir.AluOpType.mult)

                nc.sync.dma_start(wout[t, :, :], wn)
                nc.sync.dma_start(iout[t, :, :], i8)
    nc.compile()
    return nc


def _prep_inputs(x, kernel, bias):
    """Host-side shard + retile. Returns list of 8 in_maps."""
    x = np.asarray(x, dtype=np.float32)
    kernel = np.asarray(kernel, dtype=np.float32)
    bias = np.asarray(bias, dtype=np.float32)
    ks = kernel * np.float32(SW)
    kh = ks.astype(np.float16)
    kl = (ks - kh.astype(np.float32)).astype(np.float16)
    wkh = np.ascontiguousarray(
        kh.reshape(KC, 128, E).transpose(1, 0, 2))               # [p, c, e]
    wkl = np.ascontiguousarray(
        kl.reshape(KC, 128, E).transpose(1, 0, 2))
    bb = np.ascontiguousarray(np.broadcast_to(bias, (128, E)))
    in_maps = []
    for core in range(N_CORES):
        xs = x[core * TS:(core + 1) * TS] * np.float32(SX)       # [1024, 7168]
        xsh = xs.astype(np.float16)
        xsl = (xs - xsh.astype(np.float32)).astype(np.float16)
        xph = np.ascontiguousarray(
            xsh.reshape(NT, 128, KC, 128).transpose(0, 3, 2, 1))  # [t, p, c, j]
        xpl = np.ascontiguousarray(
            xsl.reshape(NT, 128, KC, 128).transpose(0, 3, 2, 1))
        in_maps.append({"xph": xph, "xpl": xpl, "wkh": wkh, "wkl": wkl,
                        "bb": bb})
    return in_maps


def kernel(x, kernel, bias, _trace=False):
    if "nc" not in _BUILt:
        _BUILt["nc"] = build_nc()
    nc = _BUILt["nc"]
    in_maps = _prep_inputs(x, kernel, bias)
    res = bass_utils.run_bass_kernel_spmd(
        nc, in_maps, core_ids=list(range(N_CORES)), trace=_trace)
    weights = np.empty((T, TOP_K), np.float32)
    idx = np.empty((T, TOP_K), np.int32)
    for core in range(N_CORES):
        weights[core * TS:(core + 1) * TS] = res.results[core]["wout"].reshape(TS, TOP_K)
        idx[core * TS:(core + 1) * TS] = (
            res.results[core]["iout"].reshape(TS, TOP_K).astype(np.int32))
    _BUILt["last_result"] = res
    return weights, idx


# revision 8
# speedup vs baseline: 1.3240x; 1.0356x over previous
"""DeepSeek-V3 router kernel for Trainium2 (8 NeuronCores, SPMD).

Computes, for x:[8192,7168] f32, kernel:[7168,256] f32, bias:[256] f32:
    scores = sigmoid(x @ kernel)
    s = scores + bias
    group top-2 sums over 8 groups of 32 -> top-4 groups mask
    top-8 experts of masked s -> idx (int32), weights = normalized gathered
    sigmoid scores * 2.5
Returns (weights:[8192,8] f32, topk_idx:[8192,8] int32).

Sharding: x split along tokens across 8 cores (1024 tokens/core); router
weight + bias replicated. Host pre-tiles x into transposed layout so the
device GEMM needs no on-chip transpose.
"""
import sys

sys.path.insert(0, "/opt/trn_rl_repo")

import numpy as np

import concourse.bass as bass
import concourse.mybir as mybir
from concourse import bacc
from concourse.tile import TileContext
from concourse import bass_utils

T, D, E = 8192, 7168, 256
N_CORES = 8
TS = T // N_CORES          # tokens per core (1024)
NT = TS // 128             # token tiles per core (8)
KC = D // 128              # contraction chunks (56)
G, EPG = 8, 32             # expert groups, experts per group
TOPK_G, TOP_K = 4, 8
SCALE = 2.5
F32 = mybir.dt.float32
F16 = mybir.dt.float16
SX, SW = 64.0, 1024.0
U32 = mybir.dt.uint32
WGRP = 8                   # kernel-chunk groups for pipelined weight loads

_BUILt = {}


def build_nc(trace_scopes=False):
    nc = bacc.Bacc(None, target_bir_lowering=False)
    xph = nc.dram_tensor("xph", [NT, 128, KC, 128], F16, kind="ExternalInput")
    xpl = nc.dram_tensor("xpl", [NT, 128, KC, 128], F16, kind="ExternalInput")
    wkh = nc.dram_tensor("wkh", [128, KC, E], F16, kind="ExternalInput")
    wkl = nc.dram_tensor("wkl", [128, KC, E], F16, kind="ExternalInput")
    bb = nc.dram_tensor("bb", [128, E], F32, kind="ExternalInput")
    wout = nc.dram_tensor("wout", [NT, 128, TOP_K], F32, kind="ExternalOutput")
    iout = nc.dram_tensor("iout", [NT, 128, TOP_K], U32, kind="ExternalOutput")

    with TileContext(nc) as tc:
        with (
            tc.tile_pool(name="const", bufs=1) as constp,
            tc.tile_pool(name="xin", bufs=2) as xinp,
            tc.tile_pool(name="ps", bufs=2, space="PSUM") as psp,
            tc.tile_pool(name="work", bufs=2) as workp,
            tc.tile_pool(name="small", bufs=2) as smallp,
        ):
            bb_sb = constp.tile([128, E], F32)
            gc = KC // WGRP  # chunks per weight group (7)
            # interleave tile-0 x loads with the weight groups so group-0
            # matmuls can start as soon as ~1.4 MB has landed
            wh_sb, wl_sb = [], []
            xh0 = xinp.tile([128, KC, 128], F16, tag="xh")
            xl0 = xinp.tile([128, KC, 128], F16, tag="xl")
            for g in range(WGRP):
                sl = slice(g * gc, (g + 1) * gc)
                wh = constp.tile([128, gc, E], F16, tag=f"wh{g}")
                wl = constp.tile([128, gc, E], F16, tag=f"wl{g}")
                nc.sync.dma_start(xh0[:, sl, :], xph[0, :, sl, :])
                nc.sync.dma_start(wh, wkh[:, sl, :])
                nc.sync.dma_start(xl0[:, sl, :], xpl[0, :, sl, :])
                nc.sync.dma_start(wl, wkl[:, sl, :])
                wh_sb.append(wh)
                wl_sb.append(wl)
            nc.sync.dma_start(bb_sb, bb[:, :])

            for t in range(NT):
                if t == 0:
                    xh, xl = xh0, xl0
                else:
                    xh = xinp.tile([128, KC, 128], F16, tag="xh")
                    xl = xinp.tile([128, KC, 128], F16, tag="xl")
                    for g in range(WGRP):
                        sl = slice(g * gc, (g + 1) * gc)
                        nc.sync.dma_start(xh[:, sl, :], xph[t, :, sl, :])
                        nc.sync.dma_start(xl[:, sl, :], xpl[t, :, sl, :])

                acc = psp.tile([128, E], F32, tag="acc")
                for c in range(KC):
                    g = c // gc
                    whc = wh_sb[g][:, c - g * gc, :]
                    wlc = wl_sb[g][:, c - g * gc, :]
                    nc.tensor.matmul(acc, xh[:, c, :], whc,
                                     start=(c == 0), stop=False)
                    nc.tensor.matmul(acc, xh[:, c, :], wlc,
                                     start=False, stop=False)
                    nc.tensor.matmul(acc, xl[:, c, :], whc,
                                     start=False, stop=(c == KC - 1))

                # sigmoid on ACT (reads PSUM, writes SBUF)
                scores = workp.tile([128, E], F32, tag="scores")
                nc.scalar.activation(scores, acc,
                                     mybir.ActivationFunctionType.Sigmoid,
                                     scale=1.0 / (SX * SW))
                # s = scores + bias
                s = workp.tile([128, E], F32, tag="s")
                nc.vector.tensor_add(s, scores, bb_sb)

                s3 = s[:].rearrange("p (g q) -> p g q", q=EPG)
                r1 = smallp.tile([128, G], F32, tag="r1")
                nc.vector.tensor_reduce(r1, s3, axis=mybir.AxisListType.X,
                                        op=mybir.AluOpType.max)
                mr = workp.tile([128, E], F32, tag="mr")
                nc.vector.match_replace(mr, r1, s, -1e30)
                r2 = smallp.tile([128, G], F32, tag="r2")
                nc.vector.tensor_reduce(
                    r2, mr[:].rearrange("p (g q) -> p g q", q=EPG),
                    axis=mybir.AxisListType.X, op=mybir.AluOpType.max)
                gs = smallp.tile([128, G], F32, tag="gs")
                nc.vector.tensor_add(gs, r1, r2)
                gs8 = smallp.tile([128, 8], F32, tag="gs8")
                nc.vector.max(gs8, gs)
                gmask = smallp.tile([128, G], F32, tag="gmask")
                nc.vector.tensor_scalar(gmask, gs, gs8[:, TOPK_G - 1:TOPK_G],
                                        None, op0=mybir.AluOpType.is_ge)
                # s_sel = s * gmask (mask broadcast over experts-per-group)
                s_sel = workp.tile([128, E], F32, tag="s_sel")
                nc.vector.tensor_mul(
                    s_sel[:].rearrange("p (g q) -> p g q", q=EPG), s3,
                    gmask[:].to_broadcast((128, G, EPG)))

                v8 = smallp.tile([128, 8], F32, tag="v8")
                nc.vector.max(v8, s_sel)
                i8 = smallp.tile([128, 8], U32, tag="i8")
                nc.vector.max_index(i8, v8, s_sel)

                # mark top-8 positions, build score array masked to them
                mark = workp.tile([128, E], F32, tag="mark")
                nc.vector.match_replace(mark, v8, s_sel, 2e30)
                hit = workp.tile([128, E], mybir.dt.uint8, tag="hit")
                nc.vector.tensor_scalar(hit, mark, 1e30, None,
                                        op0=mybir.AluOpType.is_ge)
                msc = workp.tile([128, E], F32, tag="msc")
                nc.vector.memset(msc, -1e30)
                nc.vector.copy_predicated(msc, hit, scores)
                sc8 = smallp.tile([128, 8], F32, tag="sc8")
                nc.vector.max(sc8, msc)
                isc8 = smallp.tile([128, 8], U32, tag="isc8")
                nc.vector.max_index(isc8, sc8, msc)

                # reorder sc8 (score-order) into s-rank order by index match
                i8f = smallp.tile([128, 8], F32, tag="i8f")
                nc.vector.tensor_copy(i8f, i8)
                isc8f = smallp.tile([128, 8], F32, tag="isc8f")
                nc.vector.tensor_copy(isc8f, isc8)
                terms = smallp.tile([128, 8, 8], F32, tag="terms")
                for k in range(8):
                    nc.vector.tensor_scalar(
                        terms[:, :, k], i8f, isc8f[:, k:k + 1], sc8[:, k:k + 1],
                        op0=mybir.AluOpType.is_equal, op1=mybir.AluOpType.mult)
                w8 = smallp.tile([128, 8], F32, tag="w8")
                nc.vector.tensor_reduce(w8, terms, axis=mybir.AxisListType.X,
                                        op=mybir.AluOpType.add)

                ssum = smallp.tile([128, 1], F32, tag="ssum")
                nc.vector.tensor_reduce(ssum, w8, axis=mybir.AxisListType.X,
                                        op=mybir.AluOpType.add)
                rec = smallp.tile([128, 1], F32, tag="rec")
                nc.vector.tensor_scalar(rec, ssum, 1e-20, None,
                                        op0=mybir.AluOpType.add)
                nc.vector.reciprocal(rec, rec)
                nc.vector.tensor_scalar(rec, rec, SCALE, None,
                                        op0=mybir.AluOpType.mult)
                wn = smallp.tile([128, 8], F32, tag="wn")
                nc.vector.tensor_scalar(wn, w8, rec[:, 0:1], None,
                                        op0=mybir.AluOpType.mult)

                nc.sync.dma_start(wout[t, :, :], wn)
                nc.sync.dma_start(iout[t, :, :], i8)
    nc.compile()
    return nc


def _prep_inputs(x, kernel, bias):
    """Host-side shard + retile. Returns list of 8 in_maps."""
    x = np.asarray(x, dtype=np.float32)
    kernel = np.asarray(kernel, dtype=np.float32)
    bias = np.asarray(bias, dtype=np.float32)
    ks = kernel * np.float32(SW)
    kh = ks.astype(np.float16)
    kl = (ks - kh.astype(np.float32)).astype(np.float16)
    wkh = np.ascontiguousarray(
        kh.reshape(KC, 128, E).transpose(1, 0, 2))               # [p, c, e]
    wkl = np.ascontiguousarray(
        kl.reshape(KC, 128, E).transpose(1, 0, 2))
    bb = np.ascontiguousarray(np.broadcast_to(bias, (128, E)))
    in_maps = []
    for core in range(N_CORES):
        xs = x[core * TS:(core + 1) * TS] * np.float32(SX)       # [1024, 7168]
        xsh = xs.astype(np.float16)
        xsl = (xs - xsh.astype(np.float32)).astype(np.float16)
        xph = np.ascontiguousarray(
            xsh.reshape(NT, 128, KC, 128).transpose(0, 3, 2, 1))  # [t, p, c, j]
        xpl = np.ascontiguousarray(
            xsl.reshape(NT, 128, KC, 128).transpose(0, 3, 2, 1))
        in_maps.append({"xph": xph, "xpl": xpl, "wkh": wkh, "wkl": wkl,
                        "bb": bb})
    return in_maps


def kernel(x, kernel, bias, _trace=False):
    if "nc" not in _BUILt:
        _BUILt["nc"] = build_nc()
    nc = _BUILt["nc"]
    in_maps = _prep_inputs(x, kernel, bias)
    res = bass_utils.run_bass_kernel_spmd(
        nc, in_maps, core_ids=list(range(N_CORES)), trace=_trace)
    weights = np.empty((T, TOP_K), np.float32)
    idx = np.empty((T, TOP_K), np.int32)
    for core in range(N_CORES):
        weights[core * TS:(core + 1) * TS] = res.results[core]["wout"].reshape(TS, TOP_K)
        idx[core * TS:(core + 1) * TS] = (
            res.results[core]["iout"].reshape(TS, TOP_K).astype(np.int32))
    _BUILt["last_result"] = res
    return weights, idx


# revision 9
# speedup vs baseline: 1.3272x; 1.0024x over previous
"""DeepSeek-V3 router kernel for Trainium2 (8 NeuronCores, SPMD).

Computes, for x:[8192,7168] f32, kernel:[7168,256] f32, bias:[256] f32:
    scores = sigmoid(x @ kernel)
    s = scores + bias
    group top-2 sums over 8 groups of 32 -> top-4 groups mask
    top-8 experts of masked s -> idx (int32), weights = normalized gathered
    sigmoid scores * 2.5
Returns (weights:[8192,8] f32, topk_idx:[8192,8] int32).

Sharding: x split along tokens across 8 cores (1024 tokens/core); router
weight + bias replicated. Host pre-tiles x into transposed layout so the
device GEMM needs no on-chip transpose.
"""
import sys

sys.path.insert(0, "/opt/trn_rl_repo")

import numpy as np

import concourse.bass as bass
import concourse.mybir as mybir
from concourse import bacc
from concourse.tile import TileContext
from concourse import bass_utils

T, D, E = 8192, 7168, 256
N_CORES = 8
TS = T // N_CORES          # tokens per core (1024)
NT = TS // 128             # token tiles per core (8)
KC = D // 128              # contraction chunks (56)
G, EPG = 8, 32             # expert groups, experts per group
TOPK_G, TOP_K = 4, 8
SCALE = 2.5
F32 = mybir.dt.float32
F16 = mybir.dt.float16
SX, SW = 64.0, 1024.0
U32 = mybir.dt.uint32
WGRP = 8                   # kernel-chunk groups for pipelined weight loads

_BUILt = {}


def build_nc(trace_scopes=False):
    nc = bacc.Bacc(None, target_bir_lowering=False)
    xhl = nc.dram_tensor("xhl", [NT, 128, KC, 2, 128], F16, kind="ExternalInput")
    wkh = nc.dram_tensor("wkh", [128, KC, E], F16, kind="ExternalInput")
    wkl = nc.dram_tensor("wkl", [128, KC, E], F16, kind="ExternalInput")
    bb = nc.dram_tensor("bb", [128, E], F32, kind="ExternalInput")
    wout = nc.dram_tensor("wout", [NT, 128, TOP_K], F32, kind="ExternalOutput")
    iout = nc.dram_tensor("iout", [NT, 128, TOP_K], U32, kind="ExternalOutput")

    with TileContext(nc) as tc:
        with (
            tc.tile_pool(name="const", bufs=1) as constp,
            tc.tile_pool(name="xin", bufs=2) as xinp,
            tc.tile_pool(name="ps", bufs=2, space="PSUM") as psp,
            tc.tile_pool(name="work", bufs=2) as workp,
            tc.tile_pool(name="small", bufs=2) as smallp,
        ):
            bb_sb = constp.tile([128, E], F32)
            gc = KC // WGRP  # chunks per weight group (7)
            # interleave tile-0 x loads with the weight groups so group-0
            # matmuls can start as soon as ~1.4 MB has landed
            wh_sb, wl_sb = [], []
            xt0 = xinp.tile([128, KC, 2, 128], F16, tag="xt")
            for g in range(WGRP):
                sl = slice(g * gc, (g + 1) * gc)
                wh = constp.tile([128, gc, E], F16, tag=f"wh{g}")
                wl = constp.tile([128, gc, E], F16, tag=f"wl{g}")
                nc.sync.dma_start(xt0[:, sl, :, :], xhl[0, :, sl, :, :])
                nc.sync.dma_start(wh, wkh[:, sl, :])
                nc.sync.dma_start(wl, wkl[:, sl, :])
                wh_sb.append(wh)
                wl_sb.append(wl)
            nc.sync.dma_start(bb_sb, bb[:, :])

            for t in range(NT):
                if t == 0:
                    xt = xt0
                else:
                    xt = xinp.tile([128, KC, 2, 128], F16, tag="xt")
                    for g in range(WGRP):
                        sl = slice(g * gc, (g + 1) * gc)
                        nc.sync.dma_start(xt[:, sl, :, :], xhl[t, :, sl, :, :])

                acc = psp.tile([128, E], F32, tag="acc")
                for c in range(KC):
                    g = c // gc
                    whc = wh_sb[g][:, c - g * gc, :]
                    wlc = wl_sb[g][:, c - g * gc, :]
                    nc.tensor.matmul(acc, xt[:, c, 0, :], whc,
                                     start=(c == 0), stop=False)
                    nc.tensor.matmul(acc, xt[:, c, 0, :], wlc,
                                     start=False, stop=False)
                    nc.tensor.matmul(acc, xt[:, c, 1, :], whc,
                                     start=False, stop=(c == KC - 1))

                # sigmoid on ACT (reads PSUM, writes SBUF)
                scores = workp.tile([128, E], F32, tag="scores")
                nc.scalar.activation(scores, acc,
                                     mybir.ActivationFunctionType.Sigmoid,
                                     scale=1.0 / (SX * SW))
                # s = scores + bias
                s = workp.tile([128, E], F32, tag="s")
                nc.vector.tensor_add(s, scores, bb_sb)

                s3 = s[:].rearrange("p (g q) -> p g q", q=EPG)
                r1 = smallp.tile([128, G], F32, tag="r1")
                nc.vector.tensor_reduce(r1, s3, axis=mybir.AxisListType.X,
                                        op=mybir.AluOpType.max)
                mr = workp.tile([128, E], F32, tag="mr")
                nc.vector.match_replace(mr, r1, s, -1e30)
                r2 = smallp.tile([128, G], F32, tag="r2")
                nc.vector.tensor_reduce(
                    r2, mr[:].rearrange("p (g q) -> p g q", q=EPG),
                    axis=mybir.AxisListType.X, op=mybir.AluOpType.max)
                gs = smallp.tile([128, G], F32, tag="gs")
                nc.vector.tensor_add(gs, r1, r2)
                gs8 = smallp.tile([128, 8], F32, tag="gs8")
                nc.vector.max(gs8, gs)
                gmask = smallp.tile([128, G], F32, tag="gmask")
                nc.vector.tensor_scalar(gmask, gs, gs8[:, TOPK_G - 1:TOPK_G],
                                        None, op0=mybir.AluOpType.is_ge)
                # s_sel = s * gmask (mask broadcast over experts-per-group)
                s_sel = workp.tile([128, E], F32, tag="s_sel")
                nc.vector.tensor_mul(
                    s_sel[:].rearrange("p (g q) -> p g q", q=EPG), s3,
                    gmask[:].to_broadcast((128, G, EPG)))

                v8 = smallp.tile([128, 8], F32, tag="v8")
                nc.vector.max(v8, s_sel)
                i8 = smallp.tile([128, 8], U32, tag="i8")
                nc.vector.max_index(i8, v8, s_sel)

                # mark top-8 positions, build score array masked to them
                mark = workp.tile([128, E], F32, tag="mark")
                nc.vector.match_replace(mark, v8, s_sel, 2e30)
                hit = workp.tile([128, E], mybir.dt.uint8, tag="hit")
                nc.vector.tensor_scalar(hit, mark, 1e30, None,
                                        op0=mybir.AluOpType.is_ge)
                msc = workp.tile([128, E], F32, tag="msc")
                nc.vector.memset(msc, -1e30)
                nc.vector.copy_predicated(msc, hit, scores)
                sc8 = smallp.tile([128, 8], F32, tag="sc8")
                nc.vector.max(sc8, msc)
                isc8 = smallp.tile([128, 8], U32, tag="isc8")
                nc.vector.max_index(isc8, sc8, msc)

                # reorder sc8 (score-order) into s-rank order by index match
                i8f = smallp.tile([128, 8], F32, tag="i8f")
                nc.vector.tensor_copy(i8f, i8)
                isc8f = smallp.tile([128, 8], F32, tag="isc8f")
                nc.vector.tensor_copy(isc8f, isc8)
                terms = smallp.tile([128, 8, 8], F32, tag="terms")
                for k in range(8):
                    nc.vector.tensor_scalar(
                        terms[:, :, k], i8f, isc8f[:, k:k + 1], sc8[:, k:k + 1],
                        op0=mybir.AluOpType.is_equal, op1=mybir.AluOpType.mult)
                w8 = smallp.tile([128, 8], F32, tag="w8")
                nc.vector.tensor_reduce(w8, terms, axis=mybir.AxisListType.X,
                                        op=mybir.AluOpType.add)

                ssum = smallp.tile([128, 1], F32, tag="ssum")
                nc.vector.tensor_reduce(ssum, w8, axis=mybir.AxisListType.X,
                                        op=mybir.AluOpType.add)
                rec = smallp.tile([128, 1], F32, tag="rec")
                nc.vector.tensor_scalar(rec, ssum, 1e-20, None,
                                        op0=mybir.AluOpType.add)
                nc.vector.reciprocal(rec, rec)
                nc.vector.tensor_scalar(rec, rec, SCALE, None,
                                        op0=mybir.AluOpType.mult)
                wn = smallp.tile([128, 8], F32, tag="wn")
                nc.vector.tensor_scalar(wn, w8, rec[:, 0:1], None,
                                        op0=mybir.AluOpType.mult)

                nc.sync.dma_start(wout[t, :, :], wn)
                nc.sync.dma_start(iout[t, :, :], i8)
    nc.compile()
    return nc


def _prep_inputs(x, kernel, bias):
    """Host-side shard + retile. Returns list of 8 in_maps."""
    x = np.asarray(x, dtype=np.float32)
    kernel = np.asarray(kernel, dtype=np.float32)
    bias = np.asarray(bias, dtype=np.float32)
    ks = kernel * np.float32(SW)
    kh = ks.astype(np.float16)
    kl = (ks - kh.astype(np.float32)).astype(np.float16)
    wkh = np.ascontiguousarray(
        kh.reshape(KC, 128, E).transpose(1, 0, 2))               # [p, c, e]
    wkl = np.ascontiguousarray(
        kl.reshape(KC, 128, E).transpose(1, 0, 2))
    bb = np.ascontiguousarray(np.broadcast_to(bias, (128, E)))
    in_maps = []
    for core in range(N_CORES):
        xs = x[core * TS:(core + 1) * TS] * np.float32(SX)       # [1024, 7168]
        xsh = xs.astype(np.float16)
        xsl = (xs - xsh.astype(np.float32)).astype(np.float16)
        xph = xsh.reshape(NT, 128, KC, 128).transpose(0, 3, 2, 1)  # [t, p, c, j]
        xpl = xsl.reshape(NT, 128, KC, 128).transpose(0, 3, 2, 1)
        xhl = np.ascontiguousarray(np.stack([xph, xpl], axis=3))   # [t,p,c,2,j]
        in_maps.append({"xhl": xhl, "wkh": wkh, "wkl": wkl, "bb": bb})
    return in_maps


def kernel(x, kernel, bias, _trace=False):
    if "nc" not in _BUILt:
        _BUILt["nc"] = build_nc()
    nc = _BUILt["nc"]
    in_maps = _prep_inputs(x, kernel, bias)
    res = bass_utils.run_bass_kernel_spmd(
        nc, in_maps, core_ids=list(range(N_CORES)), trace=_trace)
    weights = np.empty((T, TOP_K), np.float32)
    idx = np.empty((T, TOP_K), np.int32)
    for core in range(N_CORES):
        weights[core * TS:(core + 1) * TS] = res.results[core]["wout"].reshape(TS, TOP_K)
        idx[core * TS:(core + 1) * TS] = (
            res.results[core]["iout"].reshape(TS, TOP_K).astype(np.int32))
    _BUILt["last_result"] = res
    return weights, idx


# revision 10
# speedup vs baseline: 1.3487x; 1.0162x over previous
"""DeepSeek-V3 router kernel for Trainium2 (8 NeuronCores, SPMD).

Computes, for x:[8192,7168] f32, kernel:[7168,256] f32, bias:[256] f32:
    scores = sigmoid(x @ kernel)
    s = scores + bias
    group top-2 sums over 8 groups of 32 -> top-4 groups mask
    top-8 experts of masked s -> idx (int32), weights = normalized gathered
    sigmoid scores * 2.5
Returns (weights:[8192,8] f32, topk_idx:[8192,8] int32).

Sharding: x split along tokens across 8 cores (1024 tokens/core); router
weight + bias replicated. Host pre-tiles x into transposed layout so the
device GEMM needs no on-chip transpose.
"""
import sys

sys.path.insert(0, "/opt/trn_rl_repo")

import numpy as np

import concourse.bass as bass
import concourse.mybir as mybir
from concourse import bacc
from concourse.tile import TileContext
from concourse import bass_utils

T, D, E = 8192, 7168, 256
N_CORES = 8
TS = T // N_CORES          # tokens per core (1024)
NT = TS // 128             # token tiles per core (8)
KC = D // 128              # contraction chunks (56)
G, EPG = 8, 32             # expert groups, experts per group
TOPK_G, TOP_K = 4, 8
SCALE = 2.5
F32 = mybir.dt.float32
F16 = mybir.dt.float16
SX, SW = 64.0, 1024.0
U32 = mybir.dt.uint32
WGRP = 8                   # kernel-chunk groups for pipelined weight loads

_BUILt = {}


def build_nc(trace_scopes=False):
    nc = bacc.Bacc(None, target_bir_lowering=False)
    xhl = nc.dram_tensor("xhl", [NT, 128, KC, 2, 128], F16, kind="ExternalInput")
    wkh = nc.dram_tensor("wkh", [128, KC, E], F16, kind="ExternalInput")
    wkl = nc.dram_tensor("wkl", [128, KC, E], F16, kind="ExternalInput")
    bb = nc.dram_tensor("bb", [128, E], F32, kind="ExternalInput")
    wout = nc.dram_tensor("wout", [NT, 128, TOP_K], F32, kind="ExternalOutput")
    iout = nc.dram_tensor("iout", [NT, 128, TOP_K], U32, kind="ExternalOutput")

    with TileContext(nc) as tc:
        with (
            tc.tile_pool(name="const", bufs=1) as constp,
            tc.tile_pool(name="xin", bufs=2) as xinp,
            tc.tile_pool(name="ps", bufs=2, space="PSUM") as psp,
            tc.tile_pool(name="work", bufs=2) as workp,
            tc.tile_pool(name="small", bufs=2) as smallp,
        ):
            bb_sb = constp.tile([128, E], F32)
            gc = KC // WGRP  # chunks per weight group (7)
            # interleave tile-0 x loads with the weight groups so group-0
            # matmuls can start as soon as ~1.4 MB has landed
            wh_sb, wl_sb = [], []
            xt0 = xinp.tile([128, KC, 2, 128], F16, tag="xt")
            for g in range(WGRP):
                sl = slice(g * gc, (g + 1) * gc)
                wh = constp.tile([128, gc, E], F16, tag=f"wh{g}")
                wl = constp.tile([128, gc, E], F16, tag=f"wl{g}")
                nc.sync.dma_start(xt0[:, sl, :, :], xhl[0, :, sl, :, :])
                nc.sync.dma_start(wh, wkh[:, sl, :])
                nc.sync.dma_start(wl, wkl[:, sl, :])
                wh_sb.append(wh)
                wl_sb.append(wl)
            nc.sync.dma_start(bb_sb, bb[:, :])

            for t in range(NT):
                if t == 0:
                    xt = xt0
                else:
                    xt = xinp.tile([128, KC, 2, 128], F16, tag="xt")
                    for g in range(WGRP):
                        sl = slice(g * gc, (g + 1) * gc)
                        nc.sync.dma_start(xt[:, sl, :, :], xhl[t, :, sl, :, :])

                acc = psp.tile([128, E], F32, tag="acc")
                for c in range(KC):
                    g = c // gc
                    whc = wh_sb[g][:, c - g * gc, :]
                    wlc = wl_sb[g][:, c - g * gc, :]
                    nc.tensor.matmul(acc, xt[:, c, 0, :], whc,
                                     start=(c == 0), stop=False)
                    nc.tensor.matmul(acc, xt[:, c, 0, :], wlc,
                                     start=False, stop=False)
                    nc.tensor.matmul(acc, xt[:, c, 1, :], whc,
                                     start=False, stop=(c == KC - 1))

                # sigmoid on ACT (reads PSUM, writes SBUF)
                scores = workp.tile([128, E], F32, tag="scores")
                nc.scalar.activation(scores, acc,
                                     mybir.ActivationFunctionType.Sigmoid,
                                     scale=1.0 / (SX * SW))
                # s = scores + bias
                s = workp.tile([128, E], F32, tag="s")
                nc.vector.tensor_add(s, scores, bb_sb)

                s3 = s[:].rearrange("p (g q) -> p g q", q=EPG)
                r1 = smallp.tile([128, G], F32, tag="r1")
                nc.vector.tensor_reduce(r1, s3, axis=mybir.AxisListType.X,
                                        op=mybir.AluOpType.max)
                mr = workp.tile([128, E], F32, tag="mr")
                nc.vector.match_replace(mr, r1, s, -1e30)
                r2 = smallp.tile([128, G], F32, tag="r2")
                nc.vector.tensor_reduce(
                    r2, mr[:].rearrange("p (g q) -> p g q", q=EPG),
                    axis=mybir.AxisListType.X, op=mybir.AluOpType.max)
                gs = smallp.tile([128, G], F32, tag="gs")
                nc.vector.tensor_add(gs, r1, r2)
                gs8 = smallp.tile([128, 8], F32, tag="gs8")
                nc.vector.max(gs8, gs)
                # s_sel = (gs >= thr) * s, fused mask-build + apply
                s_sel = workp.tile([128, E], F32, tag="s_sel")
                nc.vector.scalar_tensor_tensor(
                    s_sel[:].rearrange("p (g q) -> p g q", q=EPG),
                    gs[:].to_broadcast((128, G, EPG)),
                    gs8[:, TOPK_G - 1:TOPK_G], s3,
                    op0=mybir.AluOpType.is_ge, op1=mybir.AluOpType.mult)

                v8 = smallp.tile([128, 8], F32, tag="v8")
                nc.vector.max(v8, s_sel)
                i8 = smallp.tile([128, 8], U32, tag="i8")
                nc.vector.max_index(i8, v8, s_sel)

                # mark top-8 positions; msc = scores at marks, 0 elsewhere
                mark = workp.tile([128, E], F32, tag="mark")
                nc.vector.match_replace(mark, v8, s_sel, 2e30)
                msc = workp.tile([128, E], F32, tag="msc")
                nc.vector.scalar_tensor_tensor(
                    msc, mark, 1e30, scores,
                    op0=mybir.AluOpType.is_ge, op1=mybir.AluOpType.mult)
                sc8 = smallp.tile([128, 8], F32, tag="sc8")
                nc.vector.max(sc8, msc)
                isc8 = smallp.tile([128, 8], U32, tag="isc8")
                nc.vector.max_index(isc8, sc8, msc)

                # reorder sc8 (score-order) into s-rank order by index match
                i8f = smallp.tile([128, 8], F32, tag="i8f")
                nc.vector.tensor_copy(i8f, i8)
                isc8f = smallp.tile([128, 8], F32, tag="isc8f")
                nc.vector.tensor_copy(isc8f, isc8)
                isc8f_mid = isc8f[:].rearrange(
                    "p (a k) -> p a k", a=1).to_broadcast((128, 8, 8))
                sc8_mid = sc8[:].rearrange(
                    "p (a k) -> p a k", a=1).to_broadcast((128, 8, 8))
                terms = smallp.tile([128, 8, 8], F32, tag="terms")
                nc.vector.tensor_tensor(
                    terms, i8f[:].to_broadcast((128, 8, 8)), isc8f_mid,
                    op=mybir.AluOpType.is_equal)
                nc.vector.tensor_mul(terms, terms, sc8_mid)
                w8 = smallp.tile([128, 8], F32, tag="w8")
                nc.vector.tensor_reduce(w8, terms, axis=mybir.AxisListType.X,
                                        op=mybir.AluOpType.add)

                ssum = smallp.tile([128, 1], F32, tag="ssum")
                nc.vector.tensor_reduce(ssum, w8, axis=mybir.AxisListType.X,
                                        op=mybir.AluOpType.add)
                rec = smallp.tile([128, 1], F32, tag="rec")
                nc.vector.reciprocal(rec, ssum)
                wn = smallp.tile([128, 8], F32, tag="wn")
                nc.vector.tensor_scalar(wn, w8, rec[:, 0:1], SCALE,
                                        op0=mybir.AluOpType.mult,
                                        op1=mybir.AluOpType.mult)

                nc.sync.dma_start(wout[t, :, :], wn)
                nc.sync.dma_start(iout[t, :, :], i8)
    nc.compile()
    return nc


def _prep_inputs(x, kernel, bias):
    """Host-side shard + retile. Returns list of 8 in_maps."""
    x = np.asarray(x, dtype=np.float32)
    kernel = np.asarray(kernel, dtype=np.float32)
    bias = np.asarray(bias, dtype=np.float32)
    ks = kernel * np.float32(SW)
    kh = ks.astype(np.float16)
    kl = (ks - kh.astype(np.float32)).astype(np.float16)
    wkh = np.ascontiguousarray(
        kh.reshape(KC, 128, E).transpose(1, 0, 2))               # [p, c, e]
    wkl = np.ascontiguousarray(
        kl.reshape(KC, 128, E).transpose(1, 0, 2))
    bb = np.ascontiguousarray(np.broadcast_to(bias, (128, E)))
    in_maps = []
    for core in range(N_CORES):
        xs = x[core * TS:(core + 1) * TS] * np.float32(SX)       # [1024, 7168]
        xsh = xs.astype(np.float16)
        xsl = (xs - xsh.astype(np.float32)).astype(np.float16)
        xph = xsh.reshape(NT, 128, KC, 128).transpose(0, 3, 2, 1)  # [t, p, c, j]
        xpl = xsl.reshape(NT, 128, KC, 128).transpose(0, 3, 2, 1)
        xhl = np.ascontiguousarray(np.stack([xph, xpl], axis=3))   # [t,p,c,2,j]
        in_maps.append({"xhl": xhl, "wkh": wkh, "wkl": wkl, "bb": bb})
    return in_maps


def kernel(x, kernel, bias, _trace=False):
    if "nc" not in _BUILt:
        _BUILt["nc"] = build_nc()
    nc = _BUILt["nc"]
    in_maps = _prep_inputs(x, kernel, bias)
    res = bass_utils.run_bass_kernel_spmd(
        nc, in_maps, core_ids=list(range(N_CORES)), trace=_trace)
    weights = np.empty((T, TOP_K), np.float32)
    idx = np.empty((T, TOP_K), np.int32)
    for core in range(N_CORES):
        weights[core * TS:(core + 1) * TS] = res.results[core]["wout"].reshape(TS, TOP_K)
        idx[core * TS:(core + 1) * TS] = (
            res.results[core]["iout"].reshape(TS, TOP_K).astype(np.int32))
    _BUILt["last_result"] = res
    return weights, idx


# revision 11
# speedup vs baseline: 1.3588x; 1.0074x over previous
"""DeepSeek-V3 router kernel for Trainium2 (8 NeuronCores, SPMD).

Computes, for x:[8192,7168] f32, kernel:[7168,256] f32, bias:[256] f32:
    scores = sigmoid(x @ kernel)
    s = scores + bias
    group top-2 sums over 8 groups of 32 -> top-4 groups mask
    top-8 experts of masked s -> idx (int32), weights = normalized gathered
    sigmoid scores * 2.5
Returns (weights:[8192,8] f32, topk_idx:[8192,8] int32).

Sharding: x split along tokens across 8 cores (1024 tokens/core); router
weight + bias replicated. Host pre-tiles x into transposed layout so the
device GEMM needs no on-chip transpose.
"""
import sys

sys.path.insert(0, "/opt/trn_rl_repo")

import numpy as np

import concourse.bass as bass
import concourse.mybir as mybir
from concourse import bacc
from concourse.tile import TileContext
from concourse import bass_utils

T, D, E = 8192, 7168, 256
N_CORES = 8
TS = T // N_CORES          # tokens per core (1024)
NT = TS // 128             # token tiles per core (8)
KC = D // 128              # contraction chunks (56)
G, EPG = 8, 32             # expert groups, experts per group
TOPK_G, TOP_K = 4, 8
SCALE = 2.5
F32 = mybir.dt.float32
F16 = mybir.dt.float16
SX, SW = 64.0, 1024.0
U32 = mybir.dt.uint32
WGRP = 14                  # kernel-chunk groups for pipelined weight loads

_BUILt = {}


def build_nc(trace_scopes=False):
    nc = bacc.Bacc(None, target_bir_lowering=False)
    xhl = nc.dram_tensor("xhl", [NT, 128, KC, 2, 128], F16, kind="ExternalInput")
    wkh = nc.dram_tensor("wkh", [128, KC, E], F16, kind="ExternalInput")
    wkl = nc.dram_tensor("wkl", [128, KC, E], F16, kind="ExternalInput")
    bb = nc.dram_tensor("bb", [128, E], F32, kind="ExternalInput")
    wout = nc.dram_tensor("wout", [NT, 128, TOP_K], F32, kind="ExternalOutput")
    iout = nc.dram_tensor("iout", [NT, 128, TOP_K], U32, kind="ExternalOutput")

    with TileContext(nc) as tc:
        with (
            tc.tile_pool(name="const", bufs=1) as constp,
            tc.tile_pool(name="xin", bufs=2) as xinp,
            tc.tile_pool(name="ps", bufs=2, space="PSUM") as psp,
            tc.tile_pool(name="work", bufs=2) as workp,
            tc.tile_pool(name="small", bufs=2) as smallp,
        ):
            bb_sb = constp.tile([128, E], F32)
            gc = KC // WGRP  # chunks per weight group (7)
            # interleave tile-0 x loads with the weight groups so group-0
            # matmuls can start as soon as ~1.4 MB has landed
            wh_sb, wl_sb = [], []
            # HAM warm-up: dummy matmuls while input DMAs stream, so the
            # first real matmuls run at 2.4 GHz instead of 1.2
            warm = constp.tile([128, 128], F16, tag="warm")
            nc.vector.memset(warm, 0.0)
            wacc = psp.tile([128, 128], F32, tag="wacc")
            for wi in range(36):
                nc.tensor.matmul(wacc, warm, warm, start=(wi == 0),
                                 stop=(wi == 35))

            xt0 = xinp.tile([128, KC, 2, 128], F16, tag="xt")
            for g in range(WGRP):
                sl = slice(g * gc, (g + 1) * gc)
                wh = constp.tile([128, gc, E], F16, tag=f"wh{g}")
                wl = constp.tile([128, gc, E], F16, tag=f"wl{g}")
                nc.sync.dma_start(xt0[:, sl, :, :], xhl[0, :, sl, :, :])
                nc.sync.dma_start(wh, wkh[:, sl, :])
                nc.sync.dma_start(wl, wkl[:, sl, :])
                wh_sb.append(wh)
                wl_sb.append(wl)
            nc.sync.dma_start(bb_sb, bb[:, :])

            for t in range(NT):
                if t == 0:
                    xt = xt0
                else:
                    xt = xinp.tile([128, KC, 2, 128], F16, tag="xt")
                    for g in range(WGRP):
                        sl = slice(g * gc, (g + 1) * gc)
                        nc.sync.dma_start(xt[:, sl, :, :], xhl[t, :, sl, :, :])

                acc = psp.tile([128, E], F32, tag="acc")
                for c in range(KC):
                    g = c // gc
                    whc = wh_sb[g][:, c - g * gc, :]
                    wlc = wl_sb[g][:, c - g * gc, :]
                    nc.tensor.matmul(acc, xt[:, c, 0, :], whc,
                                     start=(c == 0), stop=False)
                    nc.tensor.matmul(acc, xt[:, c, 0, :], wlc,
                                     start=False, stop=False)
                    nc.tensor.matmul(acc, xt[:, c, 1, :], whc,
                                     start=False, stop=(c == KC - 1))

                # sigmoid on ACT (reads PSUM, writes SBUF)
                scores = workp.tile([128, E], F32, tag="scores")
                nc.scalar.activation(scores, acc,
                                     mybir.ActivationFunctionType.Sigmoid,
                                     scale=1.0 / (SX * SW))
                # s = scores + bias
                s = workp.tile([128, E], F32, tag="s")
                nc.vector.tensor_add(s, scores, bb_sb)

                s3 = s[:].rearrange("p (g q) -> p g q", q=EPG)
                r1 = smallp.tile([128, G], F32, tag="r1")
                nc.vector.tensor_reduce(r1, s3, axis=mybir.AxisListType.X,
                                        op=mybir.AluOpType.max)
                mr = workp.tile([128, E], F32, tag="mr")
                nc.vector.match_replace(mr, r1, s, -1e30)
                r2 = smallp.tile([128, G], F32, tag="r2")
                nc.vector.tensor_reduce(
                    r2, mr[:].rearrange("p (g q) -> p g q", q=EPG),
                    axis=mybir.AxisListType.X, op=mybir.AluOpType.max)
                gs = smallp.tile([128, G], F32, tag="gs")
                nc.vector.tensor_add(gs, r1, r2)
                gs8 = smallp.tile([128, 8], F32, tag="gs8")
                nc.vector.max(gs8, gs)
                # s_sel = (gs >= thr) * s, fused mask-build + apply
                s_sel = workp.tile([128, E], F32, tag="s_sel")
                nc.vector.scalar_tensor_tensor(
                    s_sel[:].rearrange("p (g q) -> p g q", q=EPG),
                    gs[:].to_broadcast((128, G, EPG)),
                    gs8[:, TOPK_G - 1:TOPK_G], s3,
                    op0=mybir.AluOpType.is_ge, op1=mybir.AluOpType.mult)

                v8 = smallp.tile([128, 8], F32, tag="v8")
                nc.vector.max(v8, s_sel)
                i8 = smallp.tile([128, 8], U32, tag="i8")
                nc.vector.max_index(i8, v8, s_sel)

                # mark top-8 positions; msc = scores at marks, 0 elsewhere
                mark = workp.tile([128, E], F32, tag="mark")
                nc.vector.match_replace(mark, v8, s_sel, 2e30)
                msc = workp.tile([128, E], F32, tag="msc")
                nc.vector.scalar_tensor_tensor(
                    msc, mark, 1e30, scores,
                    op0=mybir.AluOpType.is_ge, op1=mybir.AluOpType.mult)
                sc8 = smallp.tile([128, 8], F32, tag="sc8")
                nc.vector.max(sc8, msc)
                isc8 = smallp.tile([128, 8], U32, tag="isc8")
                nc.vector.max_index(isc8, sc8, msc)

                # reorder sc8 (score-order) into s-rank order by index match
                i8f = smallp.tile([128, 8], F32, tag="i8f")
                nc.vector.tensor_copy(i8f, i8)
                isc8f = smallp.tile([128, 8], F32, tag="isc8f")
                nc.vector.tensor_copy(isc8f, isc8)
                isc8f_mid = isc8f[:].rearrange(
                    "p (a k) -> p a k", a=1).to_broadcast((128, 8, 8))
                sc8_mid = sc8[:].rearrange(
                    "p (a k) -> p a k", a=1).to_broadcast((128, 8, 8))
                terms = smallp.tile([128, 8, 8], F32, tag="terms")
                nc.vector.tensor_tensor(
                    terms, i8f[:].to_broadcast((128, 8, 8)), isc8f_mid,
                    op=mybir.AluOpType.is_equal)
                nc.vector.tensor_mul(terms, terms, sc8_mid)
                w8 = smallp.tile([128, 8], F32, tag="w8")
                nc.vector.tensor_reduce(w8, terms, axis=mybir.AxisListType.X,
                                        op=mybir.AluOpType.add)

                ssum = smallp.tile([128, 1], F32, tag="ssum")
                nc.vector.tensor_reduce(ssum, w8, axis=mybir.AxisListType.X,
                                        op=mybir.AluOpType.add)
                rec = smallp.tile([128, 1], F32, tag="rec")
                nc.vector.reciprocal(rec, ssum)
                wn = smallp.tile([128, 8], F32, tag="wn")
                nc.vector.tensor_scalar(wn, w8, rec[:, 0:1], SCALE,
                                        op0=mybir.AluOpType.mult,
                                        op1=mybir.AluOpType.mult)

                nc.sync.dma_start(wout[t, :, :], wn)
                nc.sync.dma_start(iout[t, :, :], i8)
    nc.compile()
    return nc


def _prep_inputs(x, kernel, bias):
    """Host-side shard + retile. Returns list of 8 in_maps."""
    x = np.asarray(x, dtype=np.float32)
    kernel = np.asarray(kernel, dtype=np.float32)
    bias = np.asarray(bias, dtype=np.float32)
    ks = kernel * np.float32(SW)
    kh = ks.astype(np.float16)
    kl = (ks - kh.astype(np.float32)).astype(np.float16)
    wkh = np.ascontiguousarray(
        kh.reshape(KC, 128, E).transpose(1, 0, 2))               # [p, c, e]
    wkl = np.ascontiguousarray(
        kl.reshape(KC, 128, E).transpose(1, 0, 2))
    bb = np.ascontiguousarray(np.broadcast_to(bias, (128, E)))
    in_maps = []
    for core in range(N_CORES):
        xs = x[core * TS:(core + 1) * TS] * np.float32(SX)       # [1024, 7168]
        xsh = xs.astype(np.float16)
        xsl = (xs - xsh.astype(np.float32)).astype(np.float16)
        xph = xsh.reshape(NT, 128, KC, 128).transpose(0, 3, 2, 1)  # [t, p, c, j]
        xpl = xsl.reshape(NT, 128, KC, 128).transpose(0, 3, 2, 1)
        xhl = np.ascontiguousarray(np.stack([xph, xpl], axis=3))   # [t,p,c,2,j]
        in_maps.append({"xhl": xhl, "wkh": wkh, "wkl": wkl, "bb": bb})
    return in_maps


def kernel(x, kernel, bias, _trace=False):
    if "nc" not in _BUILt:
        _BUILt["nc"] = build_nc()
    nc = _BUILt["nc"]
    in_maps = _prep_inputs(x, kernel, bias)
    res = bass_utils.run_bass_kernel_spmd(
        nc, in_maps, core_ids=list(range(N_CORES)), trace=_trace)
    weights = np.empty((T, TOP_K), np.float32)
    idx = np.empty((T, TOP_K), np.int32)
    for core in range(N_CORES):
        weights[core * TS:(core + 1) * TS] = res.results[core]["wout"].reshape(TS, TOP_K)
        idx[core * TS:(core + 1) * TS] = (
            res.results[core]["iout"].reshape(TS, TOP_K).astype(np.int32))
    _BUILt["last_result"] = res
    return weights, idx
